# revision 10
# baseline (speedup 1.0000x reference)
"""Trainium2 Bass kernel for nn_Decorrelation (spline-triangular decorrelation).

lam[n,l] = f_l(clip(x[n, cols[l]])) with f_l a uniform-knot cubic B-spline
(14 coefficients params[:,l]).  Evaluated exactly via a two-sided
truncated-power expansion in knot coordinates s = 1.1*x + 8.5 in [3,14]:

    f(s) = sum_{c=7..13} DL[c,l]*relu(s-c)^3 + sum_{c=4..10} DR[c,l]*relu(c-s)^3

(B-splines k>=7 vanish left of s=4, k<7 vanish right of s=11; splitting at
k=7 keeps |relu arg| <= 7 so fp32 error stays ~1e-6 absolute.)

Device pipeline per core (8-way data-parallel over the batch):
  DVE: s-clamp, 7 left relus, cube, products r3*D, reduce -> M rows, out=M@x
  ACT: 7 right relus, square, penalty squares (accumulate)
  GPSIMD: penalty partition reduce;  sync: DMAs.
M background (zeros + unit diag) memset once; lam slots overwritten in place.
"""

import sys

for _p in ("/opt/trn_rl_repo",):
    if _p not in sys.path:
        sys.path.insert(0, _p)

import numpy as np

DEGREE, ORDER, V = 12, 3, 10
L = V * (V - 1) // 2          # 45
NCOEF = DEGREE + ORDER - 1    # 14
LO, HI = -5.0, 5.0
D_KNOT = (HI - LO) / (DEGREE - 1)     # 10/11
K0 = 7
N_CORES = 8
N_FULL = 65536
N_SHARD = N_FULL // N_CORES   # 8192
T_TILES = N_SHARD // 128      # 64
TC = 8                        # tiles per product chunk
N_CHUNKS = T_TILES // TC      # 8

GROUPS = [(j, list(range(j + 1, V))) for j in range(V - 1)]  # (j, i-list)


def _dtable(params: np.ndarray) -> np.ndarray:
    """[14, 45] truncated-power coefficients, (j,i)-grouped column order.
    Slots 0..6: left relu(s-c)^3, c=7..13; slots 7..13: right, c=4..10."""
    p = params.astype(np.float64)
    w5 = np.array([1.0, -4.0, 6.0, -4.0, 1.0]) / 6.0
    DL = np.zeros((14, L))
    DR = np.zeros((14, L))
    for k in range(NCOEF):
        for m in range(5):
            c = k + m
            if k >= K0:
                if c <= 13:
                    DL[c] += w5[m] * p[k]
            elif c >= 4:
                DR[c] += w5[m] * p[k]
    slots = np.zeros((14, L))
    for q, c in enumerate(range(K0, 14)):
        slots[q] = DL[c]
    for q, c in enumerate(range(4, K0 + 4)):
        slots[7 + q] = DR[c]
    rows, cols = np.tril_indices(V, k=-1)
    tril_idx = {(int(i), int(j)): t for t, (i, j) in enumerate(zip(rows, cols))}
    perm = [tril_idx[(i, j)] for (j, ilist) in GROUPS for i in ilist]
    return slots[:, perm].astype(np.float32)


S_MUL = float(1.0 / D_KNOT)               # 1.1
S_ADD = float(5.0 / D_KNOT + 3.0)         # 8.5
LEFT_C = list(range(K0, 14))              # slots 0..6
RIGHT_C = list(range(4, K0 + 4))          # slots 7..13


def build_program():
    import concourse.bass as bass
    import concourse.mybir as mybir
    from contextlib import ExitStack

    f32 = mybir.dt.float32
    Alu = mybir.AluOpType
    Act = mybir.ActivationFunctionType
    Axis = mybir.AxisListType

    nc = bass.Bass("TRN2", target_bir_lowering=False, debug=False)

    x_in = nc.declare_dram_parameter("x_in", [N_SHARD, V], f32, isOutput=False)
    dt_in = nc.declare_dram_parameter("dtab", [128, 14 * L], f32, isOutput=False)
    pt_in = nc.declare_dram_parameter("params_t", [L, NCOEF], f32, isOutput=False)
    m_out = nc.declare_dram_parameter("m_out", [N_SHARD, V * V], f32, isOutput=True)
    o_out = nc.declare_dram_parameter("o_out", [N_SHARD, V], f32, isOutput=True)
    pen_out = nc.declare_dram_parameter("pen_out", [1, 4], f32, isOutput=True)

    # activation bias constants for the right-sided relus (allocated before
    # the stack-managed tensors so sbuf frees stay LIFO)
    for c in RIGHT_C:
        cb = nc.alloc_sbuf_tensor(f"constb-{c}", [128, 1], f32)
        nc.gpsimd.memset(cb.ap(), float(c))
        nc.const_aps.aps[(f32, float(c))] = cb.ap()
    nc.all_engine_barrier()

    ctx = ExitStack()
    sb = lambda name, shape: ctx.enter_context(
        nc.sbuf_tensor(name, shape, f32))

    x_sb = sb("x_sb", [128, T_TILES, V])
    s_sb = sb("s_sb", [128, T_TILES * V])
    r_sb = sb("r_sb", [128, 14, T_TILES * V])
    r3_sb = sb("r3_sb", [128, 14, T_TILES * V])
    d_sb = sb("d_sb", [128, 14, L])
    prod_sb = sb("prod_sb", [128, TC, L, 14])
    m_sb = sb("m_sb", [128, T_TILES, V * V])
    oprod_sb = sb("oprod_sb", [128, TC, V, V])
    o_sb = sb("o_sb", [128, T_TILES, V])
    pt_sb = sb("pt_sb", [L, NCOEF])
    d1_sb = sb("d1_sb", [L, NCOEF - 1])
    d2_sb = sb("d2_sb", [L, NCOEF - 2])
    psc_sb = sb("psc_sb", [L, NCOEF])
    pp_sb = sb("pp_sb", [L, 4])
    pen_sb = sb("pen_sb", [1, 4])

    sem_names = ["x_ld", "dt_ld", "pt_ld", "init", "s", "rdve", "sq", "cube",
                 "pen", "st"]
    for k in range(N_CHUNKS):
        sem_names += [f"re{k}", f"ov{k}"]
    sems = {nm: ctx.enter_context(nc.semaphore(f"sem_{nm}")) for nm in sem_names}

    with ctx, nc.Block() as block:

        @block.sync
        def _(sync):
            sync.dma_start(
                out=x_sb[:, :, :],
                in_=x_in.ap().rearrange("(p t) v -> p t v", p=128),
            ).then_inc(sems["x_ld"], 16)
            sync.dma_start(
                out=d_sb[:, :, :],
                in_=dt_in.ap().rearrange("p (c l) -> p c l", c=14),
            ).then_inc(sems["dt_ld"], 16)
            sync.dma_start(out=pt_sb[:, :], in_=pt_in.ap()).then_inc(
                sems["pt_ld"], 16)
            m_dram = m_out.ap().rearrange("(p t) f -> p t f", p=128)
            sync.wait_ge(sems["init"], 1)
            for k in range(N_CHUNKS):
                sync.wait_ge(sems[f"re{k}"], len(GROUPS))
                sync.dma_start(
                    out=m_dram[:, k * TC:(k + 1) * TC, :],
                    in_=m_sb[:, k * TC:(k + 1) * TC, :],
                ).then_inc(sems["st"], 16)
            for k in range(N_CHUNKS):
                sync.wait_ge(sems[f"ov{k}"], 1)
            sync.dma_start(
                out=o_out.ap().rearrange("(p t) v -> p t v", p=128),
                in_=o_sb[:, :, :],
            ).then_inc(sems["st"], 16)
            sync.wait_ge(sems["pen"], 1)
            sync.dma_start(out=pen_out.ap(), in_=pen_sb[:, :]).then_inc(
                sems["st"], 16)

        @block.vector
        def _(vector):
            # M background: zeros + unit diagonal
            m_flat = m_sb.ap().rearrange("p t f -> p (t f)")
            vector.memset(m_flat, 0.0)
            m_row = m_sb.ap()  # [128, T, 100], element steps
            diag = bass.AP(tensor=m_row.tensor, offset=m_row.offset,
                           ap=[m_row.ap[0], m_row.ap[1], [V + 1, V]])
            vector.memset(diag, 1.0)
            vector.memset(pen_sb[:, :], 0.0).then_inc(sems["init"], 1)

            vector.wait_ge(sems["x_ld"], 16)
            x_flat = x_sb.ap().rearrange("p t v -> p (t v)")
            vector.tensor_scalar(s_sb[:, :], x_flat, S_MUL, S_ADD,
                                 Alu.mult, Alu.add)
            vector.tensor_scalar(s_sb[:, :], s_sb[:, :], 3.0, 14.0,
                                 Alu.max, Alu.min).then_inc(sems["s"], 1)
            for q, c in enumerate(LEFT_C):     # r = max(s - c, 0)
                vector.tensor_scalar(r_sb[:, q, :], s_sb[:, :],
                                     float(c), 0.0, Alu.subtract, Alu.max)
            vector.nop().then_inc(sems["rdve"], 1)

            vector.wait_ge(sems["sq"], 1)
            r_flat = r_sb.ap().rearrange("p c f -> p (c f)")
            r3_flat = r3_sb.ap().rearrange("p c f -> p (c f)")
            vector.tensor_tensor(r3_flat, r3_flat, r_flat, Alu.mult).then_inc(
                sems["cube"], 1)

            vector.wait_ge(sems["dt_ld"], 16)
            r3v = r3_sb.ap()   # [128, 14, T*V]
            dv = d_sb.ap()     # [128, 14, L]
            mv = m_sb.ap()     # [128, T, 100]
            for k in range(N_CHUNKS):
                t0 = k * TC
                l0 = 0
                for (j, ilist) in GROUPS:
                    sz = len(ilist)
                    in0 = bass.AP(tensor=r3v.tensor,
                                  offset=r3v.offset + t0 * V + j,
                                  ap=[r3v.ap[0], [V, TC], [0, sz],
                                      [T_TILES * V, 14]])
                    in1 = bass.AP(tensor=dv.tensor, offset=dv.offset + l0,
                                  ap=[dv.ap[0], [0, TC], [1, sz], [L, 14]])
                    outp = prod_sb[:, :, l0:l0 + sz, :]
                    vector.tensor_tensor(outp, in0, in1, Alu.mult)
                    red_out = bass.AP(
                        tensor=mv.tensor,
                        offset=mv.offset + t0 * 100 + (j + 1) * V + j,
                        ap=[mv.ap[0], [100, TC], [V, sz]])
                    vector.tensor_reduce(red_out, outp, Axis.X,
                                         Alu.add).then_inc(sems[f"re{k}"], 1)
                    l0 += sz
            for k in range(N_CHUNKS):
                t0 = k * TC
                min0 = bass.AP(tensor=mv.tensor, offset=mv.offset + t0 * 100,
                               ap=[mv.ap[0], [100, TC], [V, V], [1, V]])
                xv = x_sb.ap()
                xin = bass.AP(tensor=xv.tensor, offset=xv.offset + t0 * V,
                              ap=[xv.ap[0], [V, TC], [0, V], [1, V]])
                vector.tensor_tensor(oprod_sb[:, :, :, :], min0, xin, Alu.mult)
                vector.tensor_reduce(o_sb[:, t0:t0 + TC, :],
                                     oprod_sb[:, :, :, :], Axis.X,
                                     Alu.add).then_inc(sems[f"ov{k}"], 1)
            vector.wait_ge(sems["pt_ld"], 16)
            pt = pt_sb.ap()
            vector.tensor_tensor(d1_sb[:, :], pt[:, 1:NCOEF],
                                 pt[:, 0:NCOEF - 1], Alu.subtract)
            d1 = d1_sb.ap()
            vector.tensor_tensor(d2_sb[:, :], d1[:, 1:NCOEF - 1],
                                 d1[:, 0:NCOEF - 2],
                                 Alu.subtract).then_inc(sems["s"], 1)

        @block.scalar
        def _(scalar):
            scalar.wait_ge(sems["s"], 1)
            for q, c in zip(range(7, 14), RIGHT_C):   # r = relu(c - s)
                scalar.activation(r_sb[:, q, :], s_sb[:, :], Act.Relu,
                                  bias=float(c), scale=-1.0)
            scalar.wait_ge(sems["rdve"], 1)
            r_flat = r_sb.ap().rearrange("p c f -> p (c f)")
            r3_flat = r3_sb.ap().rearrange("p c f -> p (c f)")
            scalar.activation(r3_flat, r_flat, Act.Square).then_inc(
                sems["sq"], 1)
            scalar.wait_ge(sems["s"], 2)
            scalar.activation(psc_sb[:, :], pt_sb[:, :], Act.Square,
                              accum_out=pp_sb[:, 2:3])
            scalar.activation(psc_sb[:, 0:NCOEF - 1], d1_sb[:, :], Act.Square,
                              accum_out=pp_sb[:, 1:2])
            scalar.activation(psc_sb[:, 0:NCOEF - 2], d2_sb[:, :], Act.Square,
                              accum_out=pp_sb[:, 0:1]).then_inc(sems["sq"], 1)

        @block.gpsimd
        def _(gpsimd):
            gpsimd.wait_ge(sems["sq"], 2)
            gpsimd.tensor_reduce(pen_sb[0:1, 0:3], pp_sb[:, 0:3],
                                 mybir.AxisListType.C,
                                 Alu.add).then_inc(sems["pen"], 1)

    return nc


_CACHE = {}
_PROFILE = {"on": False}


def kernel(x: np.ndarray, log_d: np.ndarray, params: np.ndarray):
    from concourse import bass_utils

    x = np.ascontiguousarray(x, dtype=np.float32)
    params_f = np.ascontiguousarray(params, dtype=np.float32)

    if "nc" not in _CACHE:
        _CACHE["nc"] = build_program()
    nc = _CACHE["nc"]

    dtab = _dtable(params_f)
    dtab_rep = np.ascontiguousarray(
        np.broadcast_to(dtab.reshape(1, 14 * L), (128, 14 * L)))
    params_t = np.ascontiguousarray(params_f.T)

    core_ids = list(range(N_CORES))
    in_maps = [{
        "x_in": x[c * N_SHARD:(c + 1) * N_SHARD],
        "dtab": dtab_rep,
        "params_t": params_t,
    } for c in core_ids]
    if _PROFILE.get("on"):
        import tempfile
        tmpdir = tempfile.mkdtemp(prefix="bass_prof_")
        res = bass_utils.run_bass_kernel_spmd(nc, in_maps, core_ids,
                                              trace=True, tmpdir=tmpdir)
        _PROFILE["exec_time_ns"] = res.exec_time_ns
        _PROFILE["mean_exec_time_ns"] = res.mean_exec_time_ns
        _PROFILE["tmpdir"] = tmpdir
    else:
        res = bass_utils.run_bass_kernel_spmd(nc, in_maps, core_ids)

    M = np.empty((N_FULL, V, V), np.float32)
    out = np.empty((N_FULL, V), np.float32)
    for c in core_ids:
        M[c * N_SHARD:(c + 1) * N_SHARD] = res.results[c]["m_out"].reshape(
            N_SHARD, V, V)
        out[c * N_SHARD:(c + 1) * N_SHARD] = res.results[c]["o_out"]
    pen = res.results[0]["pen_out"][0]
    return (out, M, np.asarray(log_d, np.float32),
            np.float32(pen[0]), np.float32(pen[1]), np.float32(pen[2]))
